# revision 33
# baseline (speedup 1.0000x reference)
"""Trainium2 Bass kernel for nn_Attention_3264175145451.

Full (unsharded) inputs in, full output out. Data-parallel over batch:
16 images / 8 cores = 2 images per core, no collectives.

Per-core pipeline (per image, n=1024 tokens, c=512, H=8 heads, d=64):
  x -> cast bf16 -> x^T (PE transpose, bf16) -> xT stored fp8 ->
  QKV projections as fp8 DoubleRow matmuls (2 kt-pair instructions
  instead of 4 bf16 ones); Q^T/K^T evacuate to bf16 (+bias), K^T
  zero-padded per head so QK^T runs as full-K=128 bf16 matmuls.
  V natural fp8 with a ones column at d=64 (96-elem row stride keeps
  the DR weight-load APs 32B-aligned).  Per head: sim^T j-tiles ->
  exp activations into a persistent per-head fp8 E buffer -> AV as
  fp8 DoubleRow pairs with M=65: row 64 of the PSUM output IS the
  softmax denominator (the ones column), so no separate denominator
  matmul -> normalize via DMA reshape, reciprocal on [64,16], DMA
  partition-broadcast -> output projection as fp8 DoubleRow (ot
  stored fp8 by the normalize multiply) + bf16 bias matmul + residual.

The two images are software-pipelined at attention-head granularity: a
queue of "filler" chunks (image-1 prep, image-0 out-proj) is drained a
few chunks per head so projection matmuls fill the PE gaps left by the
scalar-engine exp bottleneck.  Engine budget per core is roughly
Scalar(exp) > PE > DVE > GpSimd.
"""

import os
import sys
from collections import deque

sys.path.insert(0, "/opt/trn_rl_repo")

import numpy as np

import concourse.bass as bass  # noqa: F401  (engine types)
import concourse.mybir as mybir
import concourse.tile as tile
from concourse import bacc
from concourse.bass_utils import run_bass_kernel_spmd
from concourse.masks import make_identity

F32 = mybir.dt.float32
BF = mybir.dt.bfloat16
FP8 = mybir.dt.float8e4
AF = mybir.ActivationFunctionType
OP = mybir.AluOpType
DR = mybir.MatmulPerfMode.DoubleRow

B = 16           # total batch
NB = 2           # batches per core
N = 1024         # tokens per image (32*32)
C = 512          # channels
H = 8            # heads
D = 64           # head dim
VW = 96          # v_sb row stride (64 V + ones col + pad to 32B align)
NCORES = 8

FP8_PROJ = bool(int(os.environ.get("BASS_ATTN_FP8_PROJ", "1")))
FP8_OUT = bool(int(os.environ.get("BASS_ATTN_FP8_OUT", "1")))
NORM_FAST = bool(int(os.environ.get("BASS_ATTN_NORM_FAST", "1")))
EXP_SHIFT = 3.5  # exp(logit - shift); cancels in softmax normalization

TRACE = bool(int(os.environ.get("BASS_ATTN_TRACE", "0")))

_cache = {}


def _register_ntff_hook():
    """Register the axon NTFF profile hook if the image lacks antenv.axon_hooks."""
    import types

    try:
        from antenv.axon_hooks import get_axon_ntff_profile_hook  # noqa: F401
        return
    except ImportError:
        pass
    try:
        from trn_agent_boot.trn_boot import _ntff_profile_via_ctypes

        hook = _ntff_profile_via_ctypes("/opt/axon/libaxon_pjrt.so")
        mod = types.ModuleType("antenv.axon_hooks")
        mod.get_axon_ntff_profile_hook = lambda: hook
        sys.modules["antenv.axon_hooks"] = mod
    except Exception:
        pass


def build_nc():
    nc = bacc.Bacc("TRN2", target_bir_lowering=False, debug=False,
                   num_devices=NCORES)

    x_ext = nc.dram_tensor("x", [NB, N, C], F32, kind="ExternalInput").ap()
    wqkv_ext = nc.dram_tensor("w_qkv", [C, 3 * C], F32, kind="ExternalInput").ap()
    bqkv_ext = nc.dram_tensor("b_qkv", [3 * C], F32, kind="ExternalInput").ap()
    wout_ext = nc.dram_tensor("w_out", [C, C], F32, kind="ExternalInput").ap()
    bout_ext = nc.dram_tensor("b_out", [C], F32, kind="ExternalInput").ap()
    y_ext = nc.dram_tensor("y", [NB, N, C], F32, kind="ExternalOutput").ap()

    with tile.TileContext(nc) as tc:
        _body(nc, tc, x_ext, wqkv_ext, bqkv_ext, wout_ext, bout_ext, y_ext)
    nc.finalize()
    return nc


def _body(nc, tc, x_ext, wqkv_ext, bqkv_ext, wout_ext, bout_ext, y_ext):
    from contextlib import ExitStack

    P_DT = FP8 if FP8_PROJ else BF    # xT / w_qkv dtype
    O_DT = FP8 if FP8_OUT else BF     # ot / w_out dtype
    EXP_BIAS = -EXP_SHIFT

    ctx = ExitStack()
    with ctx:
        wp = ctx.enter_context(tc.tile_pool(name="wp", bufs=1))
        stp = ctx.enter_context(tc.tile_pool(name="stp", bufs=1))
        persist = ctx.enter_context(tc.tile_pool(name="persist", bufs=2))
        xnp = ctx.enter_context(tc.tile_pool(name="xnp", bufs=3))
        xbp = ctx.enter_context(tc.tile_pool(name="xbp", bufs=3))
        ep = ctx.enter_context(tc.tile_pool(name="ep", bufs=4))
        rp = ctx.enter_context(tc.tile_pool(name="rp", bufs=2))
        rbp = ctx.enter_context(tc.tile_pool(name="rbp", bufs=2))
        tbp = ctx.enter_context(tc.tile_pool(name="tbp", bufs=2))
        yp = ctx.enter_context(tc.tile_pool(name="yp", bufs=3))
        drp = ctx.enter_context(tc.tile_pool(name="drp", bufs=3, space="DRAM"))
        psq = ctx.enter_context(tc.tile_pool(name="psq", bufs=2, space="PSUM"))
        psf = ctx.enter_context(tc.tile_pool(name="psf", bufs=2, space="PSUM"))
        pso = ctx.enter_context(tc.tile_pool(name="pso", bufs=2, space="PSUM"))

        # ---- constants ----
        ident = wp.tile([128, 128], BF, tag="ident")
        make_identity(nc, ident[:])

        # warm the Exp activation table while the PE is still in prep
        scr = wp.tile([1, 2], F32, tag="scr")
        nc.vector.memset(scr[:], 0.0)
        nc.scalar.activation(out=scr[:], in_=scr[:], func=AF.Exp, scale=1.0)
        # per-partition exp bias column (the fp8 range shift)
        ebias = wp.tile([128, 1], F32, tag="ebias")
        nc.vector.memset(ebias[:], EXP_BIAS)

        # ---- weights: the wst DMAs + DVE casts gate the phase-A Q/K
        # m-tile-0 chunks and thus the first exp.  half-0 (heads 0-3)
        # dispatches on the idle scalar queue, in parallel with the x0
        # prefetch dispatches on sync.
        # w_qkv viewed [c, h, t, d]; t: 0=q, 1=k, 2=v.
        # wq/wk lhsT layout [p, kt, (h d)]: m-tile mt of Q^T/K^T covers
        # heads 2mt, 2mt+1 (head-pair partition layout).
        wq_sb = wp.tile([128, 4, C], P_DT, tag="wq")
        wk_sb = wp.tile([128, 4, C], P_DT, tag="wk")
        wv_sb = wp.tile([128, 4, C], P_DT, tag="wv")

        def load_w_half(half, q):
            for kt in range(4):
                wst = stp.tile([128, 4, 3, 64], F32, tag="wst", bufs=3)
                q.dma_start(
                    out=wst[:],
                    in_=wqkv_ext.rearrange("(kt p) (h t d) -> kt p h t d",
                                           p=128, h=H, t=3)
                    [kt, :, 4 * half:4 * half + 4])
                for w_sb, t in ((wq_sb, 0), (wk_sb, 1), (wv_sb, 2)):
                    nc.vector.tensor_copy(
                        w_sb[:, kt].rearrange(
                            "p (h d) -> p h d", h=H)[:, 4 * half:4 * half + 4],
                        wst[:, :, t, :])

        load_w_half(0, nc.scalar)

        # ---- x prefetch, both images, persistent (also the residual
        # source for out-projection: no re-DMA, no tail DMA stalls).
        # Per-tile DMAs for image 0 so the first transposes start as
        # soon as tile 0 lands; image-1 as one DMA on the gpsimd queue.
        x_pref = []
        xp0 = xnp.tile([128, 8, C], F32, tag="xp0", bufs=1, name="xp0")
        for nt in range(8):
            nc.sync.dma_start(out=xp0[:, nt, :],
                              in_=x_ext[0, bass.ts(nt, 128), :])
        x_pref.append(xp0)
        xp1 = xnp.tile([128, 8, C], F32, tag="xp1", bufs=1, name="xp1")
        x_pref.append(xp1)

        load_w_half(1, nc.sync)

        # zero row staged to DRAM for the K^T padding broadcast DMAs
        # (the zero-pads gate the first QK of each image)
        zrow = wp.tile([1, N], BF, tag="zrow")
        nc.vector.memset(zrow[:], 0.0)
        zd = drp.tile([N], BF, tag="zd")
        nc.sync.dma_start(out=zd[:], in_=zrow[:])
        _zd = zd[:]

        def emit_zpads(k_sb, q):
            k_v = k_sb[:].rearrange("p (hh two) n -> p hh two n", two=2)
            for dst in (k_v[64:128, :, 0, :], k_v[0:64, :, 1, :]):
                q.dma_start(out=dst, in_=bass.AP(
                    tensor=_zd.tensor, offset=_zd.offset,
                    ap=[[0, 64], [0, 4], [1, N]]))

        def alloc_tiles(name):
            xT = persist.tile([128, 4, N], P_DT, tag="xT", name=f"xT{name}")
            q_sb = persist.tile([128, 4, N], BF, tag="q", name=f"q{name}")
            # K^T zero-padded per head: head hh occupies rows 0-63 (even)
            # or 64-127 (odd) of k_sb[:, hh, :]; the other half is zero so
            # QK^T runs as a full-K=128 matmul against the q head pair.
            k_sb = persist.tile([128, H, N], BF, tag="k", name=f"k{name}")
            # V natural, fp8, rows padded to 96 elems (96B strides keep
            # the DR weight loads 32B-aligned); col 64 is the ones column
            # whose PSUM row is the softmax denominator.
            v_sb = persist.tile([128, 8, H, VW], FP8, tag="v", name=f"v{name}")
            nc.vector.memset(v_sb[:, :, :, D:D + 1], 1.0)
            ot = persist.tile([128, 4, N], O_DT, tag="ot", name=f"ot{name}")
            return xT, q_sb, k_sb, v_sb, ot

        tiles0 = alloc_tiles("0")
        tiles1 = alloc_tiles("1")
        emit_zpads(tiles0[2], nc.gpsimd)

        if NORM_FAST:
            # partition_broadcast is a Q7 software op in the `attn`
            # gpsimd library (the default `standard` library runs a
            # different program and produces garbage).  The ~10us Q7
            # reload stalls the gpsimd queue, so it must come after the
            # identity/zero-pad emission; the first broadcast consumer
            # (av(0,0)) is ~55us in.
            from concourse import library_config
            nc.gpsimd.load_library(library_config.attn)

        # per-partition bias columns for Q^T / K^T m-tiles: b?_col[:, mt]
        # is the bias for the 128 f-dims (heads 2mt, 2mt+1) of m-tile mt.
        bq_col = wp.tile([128, 4], F32, tag="bqc")
        bk_col = wp.tile([128, 4], F32, tag="bkc")
        bqkv_v = bqkv_ext.rearrange("(mt hp t d) -> t hp d mt",
                                    mt=4, hp=2, t=3, d=D)
        for b_col, t in ((bq_col, 0), (bk_col, 1)):
            for hp in range(2):
                nc.scalar.dma_start(out=b_col[bass.ts(hp, 64), :],
                                    in_=bqkv_v[t][hp])
        # b_v broadcast over partitions: [128, (h d)] from dram with 0-stride
        bv_bc = wp.tile([128, C], F32, tag="bvb")
        bv_src = bass.AP(tensor=bqkv_ext.tensor, offset=2 * D,
                         ap=[[0, 128], [3 * D, H], [1, D]])
        nc.sync.dma_start(out=bv_bc[:].rearrange("p (h d) -> p h d", h=H),
                          in_=bv_src)
        # b_out broadcast over partitions
        bo_bc = wp.tile([128, C], F32, tag="bob")
        bo_src = bass.AP(tensor=bout_ext.tensor, offset=0,
                         ap=[[0, 128], [1, C]])
        nc.sync.dma_start(out=bo_bc[:], in_=bo_src)
        bo_bf = wp.tile([128, C], BF, tag="bo_bf")
        nc.gpsimd.tensor_copy(bo_bf[:], bo_bc[:])
        ones128 = wp.tile([128, 128], BF, tag="ones128")
        nc.vector.memset(ones128[:], 1.0 / 128.0)
        ones_col = wp.tile([1, 128], BF, tag="ones_col")
        nc.vector.memset(ones_col[:], 1.0)

        # non-startup-critical load, after everything that gates the
        # first exp
        wo_sb = wp.tile([128, 4, C], O_DT, tag="wo")
        for kt in range(4):
            wost = stp.tile([128, C], F32, tag="wost", bufs=2)
            nc.sync.dma_start(
                out=wost[:],
                in_=wout_ext.rearrange("(kt p) f -> kt p f", p=128)[kt])
            nc.gpsimd.tensor_copy(wo_sb[:, kt], wost[:])

        # prep chunk emitters; `eng` selects who does the PSUM->SBUF
        # copies: 's' scalar (activation w/ bias), 'v' DVE, 'g' gpsimd.
        def _copy(eng, dst, src):
            if eng == 's':
                nc.scalar.copy(dst, src)
            elif eng == 'g':
                nc.gpsimd.tensor_copy(dst, src)
            else:
                nc.vector.tensor_copy(dst, src)

        def chunk_transpose(b, tiles, nt, eng):
            """x n-tile nt -> bf16 (gpsimd) -> transpose -> xT (P_DT).
            bf16 transposes run 2x the fp32 rate; the PSUM tile is a
            bitcast view of the shared fp32 pool tag."""
            xT = tiles[0]
            xn = x_pref[b][:, nt, :]
            xb = xbp.tile([128, C], BF, tag="xb")
            nc.gpsimd.tensor_copy(xb[:], xn)
            pt = psf.tile([128, C], F32, tag="f")
            ptb = pt[:, 0:C // 2].bitcast(BF)
            for ct in range(4):
                nc.tensor.transpose(ptb[:, bass.ts(ct, 128)],
                                    xb[:, bass.ts(ct, 128)], ident[:])
            src = ptb.rearrange("p (ct j) -> p ct j", ct=4)
            dst = xT[:, :, bass.ts(nt, 128)]
            _copy(eng, dst, src)

        def chunk_v(b, tiles, it):
            """V natural for i-tile it (+bias via broadcast add on DVE)."""
            xT, _, _, v_sb, _ = tiles
            pv = psf.tile([128, C], F32, tag="f")
            if FP8_PROJ:
                for t in range(2):
                    nc.tensor.matmul(pv[:],
                                     xT[:, 2 * t:2 * t + 2, bass.ts(it, 128)],
                                     wv_sb[:, 2 * t:2 * t + 2, :],
                                     start=(t == 0), stop=(t == 1),
                                     perf_mode=DR)
            else:
                for kt in range(4):
                    nc.tensor.matmul(pv[:], xT[:, kt, bass.ts(it, 128)],
                                     wv_sb[:, kt, :],
                                     start=(kt == 0), stop=(kt == 3))
            nc.vector.tensor_tensor(
                v_sb[:, it, :, 0:D],
                pv[:].rearrange("p (h d) -> p h d", h=H),
                bv_bc[:].rearrange("p (h d) -> p h d", h=H), op=OP.add)

        def chunk_qk(b, tiles, mt, t, ih, eng):
            """Q^T (t=0) / K^T (t=1) m-tile mt, query half ih, + bias."""
            xT, q_sb, k_sb = tiles[0], tiles[1], tiles[2]
            w_sb, b_col = (wq_sb, bq_col) if t == 0 else (wk_sb, bk_col)
            pq = psf.tile([128, C], F32, tag="f")
            isl = bass.ts(ih, 512)
            if FP8_PROJ:
                for u in range(2):
                    nc.tensor.matmul(pq[:],
                                     w_sb[:, 2 * u:2 * u + 2, bass.ts(mt, 128)],
                                     xT[:, 2 * u:2 * u + 2, isl],
                                     start=(u == 0), stop=(u == 1),
                                     perf_mode=DR)
            else:
                for kt in range(4):
                    nc.tensor.matmul(pq[:], w_sb[:, kt, bass.ts(mt, 128)],
                                     xT[:, kt, isl],
                                     start=(kt == 0), stop=(kt == 3))
            if t == 0:
                if eng == 's':
                    nc.scalar.activation(out=q_sb[:, mt, isl], in_=pq[:],
                                         func=AF.Identity,
                                         bias=b_col[:, mt:mt + 1])
                else:
                    nc.vector.tensor_scalar_add(q_sb[:, mt, isl], pq[:],
                                                b_col[:, mt:mt + 1])
            else:
                # K^T: split the head pair into its zero-padded slots
                dsts = (k_sb[0:64, 2 * mt, isl], k_sb[64:128, 2 * mt + 1, isl])
                for hp, dst in enumerate(dsts):
                    psl = slice(64 * hp, 64 * hp + 64)
                    if eng == 's':
                        nc.scalar.activation(out=dst, in_=pq[psl, :],
                                             func=AF.Identity,
                                             bias=b_col[psl, mt:mt + 1])
                    else:
                        nc.vector.tensor_scalar_add(dst, pq[psl, :],
                                                    b_col[psl, mt:mt + 1])

        def attn_head(b, tiles, hh, fill, slots):
            """QK^T + exp for one head.  AV/normalize is NOT emitted here;
            the caller queues it as filler chunks so it drains inside the
            NEXT head's PE gap slots instead of blocking the exp stream at
            the head boundary (the PE executes its queue in order)."""
            _, q_sb, k_sb, v_sb, ot = tiles
            g = hh // 2
            e_t = ep.tile([128, 8, N], FP8, tag="E")
            for jt in range(8):
                pq = psq.tile([128, N], F32, tag="qk")
                for ih in range(2):
                    nc.tensor.matmul(pq[:, bass.ts(ih, 512)],
                                     k_sb[:, hh, bass.ts(jt, 128)],
                                     q_sb[:, g, bass.ts(ih, 512)],
                                     start=True, stop=True)
                nc.scalar.activation(out=e_t[:, jt, :], in_=pq[:],
                                     func=AF.Exp, scale=0.125, bias=ebias[:])
                if jt in slots:
                    fill()
            return e_t

        def av_chunk(b, tiles, hh, ih, e_t, tbb):
            """AV + normalize for one (head, query-half); ~1 PE group."""
            v_sb, ot = tiles[3], tiles[4]
            g = hh // 2
            po = pso.tile([128, 512], F32, tag="o")
            isl = bass.ts(ih, 512)
            # AV DoubleRow pairs; M=65: the ones column of V makes
            # PSUM row 64 the softmax denominator for this half.
            for t in range(4):
                nc.tensor.matmul(po[0:D + 1, :],
                                 v_sb[:, 2 * t:2 * t + 2, hh, 0:D + 1],
                                 e_t[:, 2 * t:2 * t + 2, isl],
                                 start=(t == 0), stop=(t == 3),
                                 perf_mode=DR)
            if NORM_FAST:
                # denominator row -> SBUF (custom-DVE ops misread PSUM at
                # base partition 64), approx reciprocal (~51 ULP), then a
                # gpsimd partition-broadcast: no DMA hops
                s_half = rp.tile([1, 512], F32, tag="s_half", bufs=4)
                nc.vector.tensor_copy(s_half[:], po[D:D + 1, :])
                rs = rp.tile([1, 512], F32, tag="rs", bufs=4)
                nc.vector.reciprocal_approx_fast(out=rs[:], in_=s_half[:])
                rbh = rbp.tile([64, 512], F32, tag="rbh", bufs=4)
                nc.gpsimd.partition_broadcast(rbh[:], rs[:], channels=64)
            else:
                s_half = rp.tile([1, 512], F32, tag="s_half", bufs=4)
                nc.vector.tensor_copy(s_half[:], po[D:D + 1, :])
                # normalize chain: reshape via DRAM, reciprocal on
                # [64, 8], broadcast back
                sdh = drp.tile([512], F32, tag="sdh", bufs=4)
                nc.sync.dma_start(out=sdh[:], in_=s_half[:])
                sph = rp.tile([64, 8], F32, tag="sph", bufs=4)
                nc.sync.dma_start(out=sph[:],
                                  in_=sdh[:].rearrange("(p f) -> p f", p=64))
                rsph = rp.tile([64, 8], F32, tag="rsph", bufs=4)
                nc.vector.reciprocal(out=rsph[:], in_=sph[:])
                rdh = drp.tile([512], F32, tag="rdh", bufs=4)
                nc.sync.dma_start(out=rdh[:].rearrange("(p f) -> p f", p=64),
                                  in_=rsph[:])
                _rdh = rdh[:]
                rbh = rbp.tile([64, 512], F32, tag="rbh", bufs=4)
                nc.sync.dma_start(out=rbh[:], in_=bass.AP(
                    tensor=_rdh.tensor, offset=_rdh.offset,
                    ap=[[0, 64], [1, 512]]))
            if hh % 2 == 0:
                nc.vector.tensor_tensor(ot[0:64, g, isl], po[0:D, :],
                                        rbh[:], op=OP.mult)
            else:
                if ih == 0:
                    tbb[0] = tbp.tile([64, N], O_DT, tag="tb", name="tb")
                nc.vector.tensor_tensor(tbb[0][:, isl], po[0:D, :],
                                        rbh[:], op=OP.mult)
                if ih == 1:
                    nc.sync.dma_start(out=ot[64:128, g, :], in_=tbb[0][:])

        def chunk_outproj(b, tiles, it, dq=None):
            # dq: DMA dispatch queue; phase D uses the scalar queue (idle
            # after the last exp) so the tail's y DMAs don't serialize
            # behind the normalize-chain hops on sync
            dq = dq or nc.sync
            ot = tiles[4]
            py = psf.tile([128, C], F32, tag="f")
            nc.tensor.matmul(py[:], ones128[:], bo_bf[:],
                             start=True, stop=False)
            if FP8_OUT:
                for u in range(2):
                    nc.tensor.matmul(py[:],
                                     ot[:, 2 * u:2 * u + 2, bass.ts(it, 128)],
                                     wo_sb[:, 2 * u:2 * u + 2, :],
                                     start=False, stop=(u == 1),
                                     perf_mode=DR)
            else:
                for g in range(4):
                    nc.tensor.matmul(py[:], ot[:, g, bass.ts(it, 128)],
                                     wo_sb[:, g, :], start=False, stop=(g == 3))
            yt = yp.tile([128, C], F32, tag="y")
            nc.vector.tensor_tensor(yt[:], py[:], x_pref[b][:, it, :],
                              op=OP.add)
            dq.dma_start(out=y_ext[b, bass.ts(it, 128), :], in_=yt[:])

        # ---------- schedule ----------
        # phase A: image-0 transposes (DVE evacs; every scalar op here
        # delays the first exp) + first Q/K group
        for nt in range(8):
            chunk_transpose(0, tiles0, nt, 'v')
        for ih in range(2):
            chunk_qk(0, tiles0, 0, 0, ih, 's')
            chunk_qk(0, tiles0, 0, 1, ih, 's')
        # image-1 x prefetch: dispatched here on the scalar queue so its
        # 2MB transfer starts only after the phase-A critical DMAs
        nc.scalar.dma_start(out=xp1[:],
                            in_=x_ext[1].rearrange("(nt p) c -> p nt c",
                                                   p=128))
        emit_zpads(tiles1[2], nc.scalar)

        # filler units, each ~1 PE accumulation group, drained a few per
        # attention head so projection/AV matmuls fill the PE idle slots
        # left while QK waits on the scalar exp drain (the PE executes
        # its queue in order, so the interleave must be finer than a
        # head).  Queue order encodes the data dependencies:
        #   v0 fully drains in head (0,0)'s 8 slots, before av(0,0) pops;
        #   qk0 m-tile mt drains before attn(0, 2mt) needs it;
        #   T1 -> qk1 -> v1 precede attention(1)/av(1) (FIFO);
        #   av(0,*) jump the queue (appendleft) so e_t tiles recycle
        #   quickly; av(1,*) append behind v1 (they read all of v_sb(1)).
        fillers = deque()
        for it in range(8):
            fillers.append(lambda it=it: chunk_v(0, tiles0, it))
        for mt in range(1, 4):
            for t in range(2):
                for ih in range(2):
                    fillers.append(
                        lambda mt=mt, t=t, ih=ih: chunk_qk(0, tiles0, mt, t, ih, 'v'))
        for nt in range(8):
            fillers.append(
                lambda nt=nt: chunk_transpose(1, tiles1, nt, 'v'))
        for mt in range(4):
            for t in range(2):
                for ih in range(2):
                    fillers.append(
                        lambda mt=mt, t=t, ih=ih: chunk_qk(1, tiles1, mt, t, ih, 'v'))
        for it in range(8):
            fillers.append(lambda it=it: chunk_v(1, tiles1, it))

        def fill():
            if fillers:
                fillers.popleft()()

        tiles_by_b = (tiles0, tiles1)
        ALL_SLOTS = frozenset(range(8))
        MOST_SLOTS = frozenset((1, 2, 3, 5, 6, 7))

        for bb in range(2):
            for hh in range(8):
                if (bb, hh) == (1, 0):
                    # image-0 out-projection drains inside attention(1)
                    for it in range(8):
                        fillers.append(
                            lambda it=it: chunk_outproj(0, tiles0, it))
                slots = (ALL_SLOTS if (bb, hh) in ((0, 0), (1, 6), (1, 7))
                         else MOST_SLOTS)
                e_t = attn_head(bb, tiles_by_b[bb], hh, fill, slots)
                tbb = [None]
                avs = [
                    lambda ih=ih, bb=bb, hh=hh, e_t=e_t, tbb=tbb: av_chunk(
                        bb, tiles_by_b[bb], hh, ih, e_t, tbb)
                    for ih in range(2)]
                if bb == 0 or hh >= 4:
                    # (v_sb(1) is fully drained by image-1 head 3, so
                    # late av chunks can safely jump the queue)
                    fillers.appendleft(avs[1])
                    fillers.appendleft(avs[0])
                else:
                    fillers.extend(avs)
        while fillers:
            fillers.popleft()()

        # phase D: image-1 out-projection
        for it in range(8):
            chunk_outproj(1, tiles1, it, dq=nc.scalar)


def kernel(x, w_qkv, b_qkv, w_out, b_out):

    x = np.ascontiguousarray(np.asarray(x, dtype=np.float32))
    w_qkv = np.ascontiguousarray(np.asarray(w_qkv, dtype=np.float32))
    b_qkv = np.ascontiguousarray(np.asarray(b_qkv, dtype=np.float32))
    w_out = np.ascontiguousarray(np.asarray(w_out, dtype=np.float32))
    b_out = np.ascontiguousarray(np.asarray(b_out, dtype=np.float32))

    bsz, hh, ww, c = x.shape
    assert (bsz, hh, ww, c) == (B, 32, 32, C)
    x_flat = x.reshape(B, N, C)

    if "nc" not in _cache:
        _cache["nc"] = build_nc()
    nc = _cache["nc"]

    if TRACE:
        _register_ntff_hook()

    in_maps = []
    for core in range(NCORES):
        in_maps.append({
            "x": x_flat[NB * core:NB * (core + 1)],
            "w_qkv": w_qkv,
            "b_qkv": b_qkv,
            "w_out": w_out,
            "b_out": b_out,
        })
    res = run_bass_kernel_spmd(nc, in_maps, list(range(NCORES)), trace=TRACE)
    _cache["last_result"] = res
    y = np.concatenate([res.results[i]["y"] for i in range(NCORES)], axis=0)
    return y.reshape(B, 32, 32, C)


# revision 34
# speedup vs baseline: 1.0847x; 1.0847x over previous
"""Trainium2 Bass kernel for nn_Attention_3264175145451.

Full (unsharded) inputs in, full output out. Data-parallel over batch:
16 images / 8 cores = 2 images per core, no collectives.

Per-core pipeline (per image, n=1024 tokens, c=512, H=8 heads, d=64):
  x -> cast bf16 -> x^T (PE transpose, bf16) -> xT stored fp8 ->
  QKV projections as fp8 DoubleRow matmuls (2 kt-pair instructions
  instead of 4 bf16 ones); Q^T/K^T evacuate to bf16 (+bias), K^T
  zero-padded per head so QK^T runs as full-K=128 bf16 matmuls.
  V natural fp8 with a ones column at d=64 (96-elem row stride keeps
  the DR weight-load APs 32B-aligned).  Per head: sim^T j-tiles ->
  exp activations into a persistent per-head fp8 E buffer -> AV as
  fp8 DoubleRow pairs with M=65: row 64 of the PSUM output IS the
  softmax denominator (the ones column), so no separate denominator
  matmul -> normalize via DMA reshape, reciprocal on [64,16], DMA
  partition-broadcast -> output projection as fp8 DoubleRow (ot
  stored fp8 by the normalize multiply) + bf16 bias matmul + residual.

The two images are software-pipelined at attention-head granularity: a
queue of "filler" chunks (image-1 prep, image-0 out-proj) is drained a
few chunks per head so projection matmuls fill the PE gaps left by the
scalar-engine exp bottleneck.  Engine budget per core is roughly
Scalar(exp) > PE > DVE > GpSimd.
"""

import os
import sys
from collections import deque

sys.path.insert(0, "/opt/trn_rl_repo")

import numpy as np

import concourse.bass as bass  # noqa: F401  (engine types)
import concourse.mybir as mybir
import concourse.tile as tile
from concourse import bacc
from concourse.bass_utils import run_bass_kernel_spmd
from concourse.masks import make_identity

F32 = mybir.dt.float32
BF = mybir.dt.bfloat16
FP8 = mybir.dt.float8e4
AF = mybir.ActivationFunctionType
OP = mybir.AluOpType
DR = mybir.MatmulPerfMode.DoubleRow

B = 16           # total batch
NB = 2           # batches per core
N = 1024         # tokens per image (32*32)
C = 512          # channels
H = 8            # heads
D = 64           # head dim
VW = 96          # v_sb row stride (64 V + ones col + pad to 32B align)
NCORES = 8

FP8_PROJ = bool(int(os.environ.get("BASS_ATTN_FP8_PROJ", "1")))
FP8_OUT = bool(int(os.environ.get("BASS_ATTN_FP8_OUT", "1")))
NORM_FAST = bool(int(os.environ.get("BASS_ATTN_NORM_FAST", "1")))
EXP_SHIFT = 3.5  # exp(logit - shift); cancels in softmax normalization

TRACE = bool(int(os.environ.get("BASS_ATTN_TRACE", "0")))

_cache = {}


def _register_ntff_hook():
    """Register the axon NTFF profile hook if the image lacks antenv.axon_hooks."""
    import types

    try:
        from antenv.axon_hooks import get_axon_ntff_profile_hook  # noqa: F401
        return
    except ImportError:
        pass
    try:
        from trn_agent_boot.trn_boot import _ntff_profile_via_ctypes

        hook = _ntff_profile_via_ctypes("/opt/axon/libaxon_pjrt.so")
        mod = types.ModuleType("antenv.axon_hooks")
        mod.get_axon_ntff_profile_hook = lambda: hook
        sys.modules["antenv.axon_hooks"] = mod
    except Exception:
        pass


def build_nc():
    nc = bacc.Bacc("TRN2", target_bir_lowering=False, debug=False,
                   num_devices=NCORES)

    x_ext = nc.dram_tensor("x", [NB, N, C], F32, kind="ExternalInput").ap()
    wqkv_ext = nc.dram_tensor("w_qkv", [C, 3 * C], F32, kind="ExternalInput").ap()
    bqkv_ext = nc.dram_tensor("b_qkv", [3 * C], F32, kind="ExternalInput").ap()
    wout_ext = nc.dram_tensor("w_out", [C, C], F32, kind="ExternalInput").ap()
    bout_ext = nc.dram_tensor("b_out", [C], F32, kind="ExternalInput").ap()
    y_ext = nc.dram_tensor("y", [NB, N, C], F32, kind="ExternalOutput").ap()

    with tile.TileContext(nc) as tc:
        _body(nc, tc, x_ext, wqkv_ext, bqkv_ext, wout_ext, bout_ext, y_ext)
    nc.finalize()
    return nc


def _body(nc, tc, x_ext, wqkv_ext, bqkv_ext, wout_ext, bout_ext, y_ext):
    from contextlib import ExitStack

    P_DT = FP8 if FP8_PROJ else BF    # xT / w_qkv dtype
    O_DT = FP8 if FP8_OUT else BF     # ot / w_out dtype
    EXP_BIAS = -EXP_SHIFT

    ctx = ExitStack()
    with ctx:
        wp = ctx.enter_context(tc.tile_pool(name="wp", bufs=1))
        stp = ctx.enter_context(tc.tile_pool(name="stp", bufs=1))
        persist = ctx.enter_context(tc.tile_pool(name="persist", bufs=2))
        xnp = ctx.enter_context(tc.tile_pool(name="xnp", bufs=3))
        ep = ctx.enter_context(tc.tile_pool(name="ep", bufs=4))
        rp = ctx.enter_context(tc.tile_pool(name="rp", bufs=2))
        rbp = ctx.enter_context(tc.tile_pool(name="rbp", bufs=2))
        tbp = ctx.enter_context(tc.tile_pool(name="tbp", bufs=2))
        yp = ctx.enter_context(tc.tile_pool(name="yp", bufs=3))
        drp = ctx.enter_context(tc.tile_pool(name="drp", bufs=3, space="DRAM"))
        psq = ctx.enter_context(tc.tile_pool(name="psq", bufs=2, space="PSUM"))
        psf = ctx.enter_context(tc.tile_pool(name="psf", bufs=2, space="PSUM"))
        pso = ctx.enter_context(tc.tile_pool(name="pso", bufs=2, space="PSUM"))

        # ---- constants ----
        ident = wp.tile([128, 128], F32, tag="ident")
        make_identity(nc, ident[:])

        # warm the Exp activation table while the PE is still in prep
        scr = wp.tile([1, 2], F32, tag="scr")
        nc.vector.memset(scr[:], 0.0)
        nc.scalar.activation(out=scr[:], in_=scr[:], func=AF.Exp, scale=1.0)
        # per-partition exp bias column (the fp8 range shift)
        ebias = wp.tile([128, 1], F32, tag="ebias")
        nc.vector.memset(ebias[:], EXP_BIAS)

        # ---- weights: the wst DMAs + DVE casts gate the phase-A Q/K
        # m-tile-0 chunks and thus the first exp.  half-0 (heads 0-3)
        # dispatches on the idle scalar queue, in parallel with the x0
        # prefetch dispatches on sync.
        # w_qkv viewed [c, h, t, d]; t: 0=q, 1=k, 2=v.
        # wq/wk lhsT layout [p, kt, (h d)]: m-tile mt of Q^T/K^T covers
        # heads 2mt, 2mt+1 (head-pair partition layout).
        wq_sb = wp.tile([128, 4, C], P_DT, tag="wq")
        wk_sb = wp.tile([128, 4, C], P_DT, tag="wk")
        wv_sb = wp.tile([128, 4, C], P_DT, tag="wv")

        def load_w_half(half, q):
            for kt in range(4):
                wst = stp.tile([128, 4, 3, 64], F32, tag="wst", bufs=3)
                q.dma_start(
                    out=wst[:],
                    in_=wqkv_ext.rearrange("(kt p) (h t d) -> kt p h t d",
                                           p=128, h=H, t=3)
                    [kt, :, 4 * half:4 * half + 4])
                for w_sb, t in ((wq_sb, 0), (wk_sb, 1), (wv_sb, 2)):
                    nc.vector.tensor_copy(
                        w_sb[:, kt].rearrange(
                            "p (h d) -> p h d", h=H)[:, 4 * half:4 * half + 4],
                        wst[:, :, t, :])

        load_w_half(0, nc.scalar)

        # ---- x prefetch, both images, persistent (also the residual
        # source for out-projection: no re-DMA, no tail DMA stalls).
        # Per-tile DMAs for image 0 so the first transposes start as
        # soon as tile 0 lands; image-1 as one DMA on the gpsimd queue.
        x_pref = []
        xp0 = xnp.tile([128, 8, C], F32, tag="xp0", bufs=1, name="xp0")
        for nt in range(8):
            nc.sync.dma_start(out=xp0[:, nt, :],
                              in_=x_ext[0, bass.ts(nt, 128), :])
        x_pref.append(xp0)
        xp1 = xnp.tile([128, 8, C], F32, tag="xp1", bufs=1, name="xp1")
        x_pref.append(xp1)

        load_w_half(1, nc.sync)

        # zero row staged to DRAM for the K^T padding broadcast DMAs
        # (the zero-pads gate the first QK of each image)
        zrow = wp.tile([1, N], BF, tag="zrow")
        nc.vector.memset(zrow[:], 0.0)
        zd = drp.tile([N], BF, tag="zd")
        nc.sync.dma_start(out=zd[:], in_=zrow[:])
        _zd = zd[:]

        def emit_zpads(k_sb, q):
            k_v = k_sb[:].rearrange("p (hh two) n -> p hh two n", two=2)
            for dst in (k_v[64:128, :, 0, :], k_v[0:64, :, 1, :]):
                q.dma_start(out=dst, in_=bass.AP(
                    tensor=_zd.tensor, offset=_zd.offset,
                    ap=[[0, 64], [0, 4], [1, N]]))

        def alloc_tiles(name):
            xT = persist.tile([128, 4, N], P_DT, tag="xT", name=f"xT{name}")
            q_sb = persist.tile([128, 4, N], BF, tag="q", name=f"q{name}")
            # K^T zero-padded per head: head hh occupies rows 0-63 (even)
            # or 64-127 (odd) of k_sb[:, hh, :]; the other half is zero so
            # QK^T runs as a full-K=128 matmul against the q head pair.
            k_sb = persist.tile([128, H, N], BF, tag="k", name=f"k{name}")
            # V natural, fp8, rows padded to 96 elems (96B strides keep
            # the DR weight loads 32B-aligned); col 64 is the ones column
            # whose PSUM row is the softmax denominator.
            v_sb = persist.tile([128, 8, H, VW], FP8, tag="v", name=f"v{name}")
            nc.vector.memset(v_sb[:, :, :, D:D + 1], 1.0)
            ot = persist.tile([128, 4, N], O_DT, tag="ot", name=f"ot{name}")
            return xT, q_sb, k_sb, v_sb, ot

        tiles0 = alloc_tiles("0")
        tiles1 = alloc_tiles("1")
        emit_zpads(tiles0[2], nc.gpsimd)

        if NORM_FAST:
            # partition_broadcast is a Q7 software op in the `attn`
            # gpsimd library (the default `standard` library runs a
            # different program and produces garbage).  The ~10us Q7
            # reload stalls the gpsimd queue, so it must come after the
            # identity/zero-pad emission; the first broadcast consumer
            # (av(0,0)) is ~55us in.
            from concourse import library_config
            nc.gpsimd.load_library(library_config.attn)

        # per-partition bias columns for Q^T / K^T m-tiles: b?_col[:, mt]
        # is the bias for the 128 f-dims (heads 2mt, 2mt+1) of m-tile mt.
        bq_col = wp.tile([128, 4], F32, tag="bqc")
        bk_col = wp.tile([128, 4], F32, tag="bkc")
        bqkv_v = bqkv_ext.rearrange("(mt hp t d) -> t hp d mt",
                                    mt=4, hp=2, t=3, d=D)
        for b_col, t in ((bq_col, 0), (bk_col, 1)):
            for hp in range(2):
                nc.scalar.dma_start(out=b_col[bass.ts(hp, 64), :],
                                    in_=bqkv_v[t][hp])
        # b_v broadcast over partitions: [128, (h d)] from dram with 0-stride
        bv_bc = wp.tile([128, C], F32, tag="bvb")
        bv_src = bass.AP(tensor=bqkv_ext.tensor, offset=2 * D,
                         ap=[[0, 128], [3 * D, H], [1, D]])
        nc.sync.dma_start(out=bv_bc[:].rearrange("p (h d) -> p h d", h=H),
                          in_=bv_src)
        # b_out broadcast over partitions
        bo_bc = wp.tile([128, C], F32, tag="bob")
        bo_src = bass.AP(tensor=bout_ext.tensor, offset=0,
                         ap=[[0, 128], [1, C]])
        nc.sync.dma_start(out=bo_bc[:], in_=bo_src)
        bo_bf = wp.tile([128, C], BF, tag="bo_bf")
        nc.gpsimd.tensor_copy(bo_bf[:], bo_bc[:])
        ones128 = wp.tile([128, 128], BF, tag="ones128")
        nc.vector.memset(ones128[:], 1.0 / 128.0)
        ones_col = wp.tile([1, 128], BF, tag="ones_col")
        nc.vector.memset(ones_col[:], 1.0)

        # non-startup-critical load, after everything that gates the
        # first exp
        wo_sb = wp.tile([128, 4, C], O_DT, tag="wo")
        for kt in range(4):
            wost = stp.tile([128, C], F32, tag="wost", bufs=2)
            nc.sync.dma_start(
                out=wost[:],
                in_=wout_ext.rearrange("(kt p) f -> kt p f", p=128)[kt])
            nc.gpsimd.tensor_copy(wo_sb[:, kt], wost[:])

        # prep chunk emitters; `eng` selects who does the PSUM->SBUF
        # copies: 's' scalar (activation w/ bias), 'v' DVE, 'g' gpsimd.
        def _copy(eng, dst, src):
            if eng == 's':
                nc.scalar.copy(dst, src)
            elif eng == 'g':
                nc.gpsimd.tensor_copy(dst, src)
            else:
                nc.vector.tensor_copy(dst, src)

        def chunk_transpose(b, tiles, nt, eng):
            """x n-tile nt -> xT columns (P_DT)."""
            xT = tiles[0]
            xn = x_pref[b][:, nt, :]
            pt = psf.tile([128, C], F32, tag="f")
            for ct in range(4):
                nc.tensor.transpose(pt[:, bass.ts(ct, 128)],
                                    xn[:, bass.ts(ct, 128)], ident[:])
            src = pt[:].rearrange("p (ct j) -> p ct j", ct=4)
            dst = xT[:, :, bass.ts(nt, 128)]
            _copy(eng, dst, src)

        def chunk_v(b, tiles, it):
            """V natural for i-tile it (+bias via broadcast add on DVE)."""
            xT, _, _, v_sb, _ = tiles
            pv = psf.tile([128, C], F32, tag="f")
            if FP8_PROJ:
                for t in range(2):
                    nc.tensor.matmul(pv[:],
                                     xT[:, 2 * t:2 * t + 2, bass.ts(it, 128)],
                                     wv_sb[:, 2 * t:2 * t + 2, :],
                                     start=(t == 0), stop=(t == 1),
                                     perf_mode=DR)
            else:
                for kt in range(4):
                    nc.tensor.matmul(pv[:], xT[:, kt, bass.ts(it, 128)],
                                     wv_sb[:, kt, :],
                                     start=(kt == 0), stop=(kt == 3))
            nc.vector.tensor_tensor(
                v_sb[:, it, :, 0:D],
                pv[:].rearrange("p (h d) -> p h d", h=H),
                bv_bc[:].rearrange("p (h d) -> p h d", h=H), op=OP.add)

        def chunk_qk(b, tiles, mt, t, ih, eng):
            """Q^T (t=0) / K^T (t=1) m-tile mt, query half ih, + bias."""
            xT, q_sb, k_sb = tiles[0], tiles[1], tiles[2]
            w_sb, b_col = (wq_sb, bq_col) if t == 0 else (wk_sb, bk_col)
            pq = psf.tile([128, C], F32, tag="f")
            isl = bass.ts(ih, 512)
            if FP8_PROJ:
                for u in range(2):
                    nc.tensor.matmul(pq[:],
                                     w_sb[:, 2 * u:2 * u + 2, bass.ts(mt, 128)],
                                     xT[:, 2 * u:2 * u + 2, isl],
                                     start=(u == 0), stop=(u == 1),
                                     perf_mode=DR)
            else:
                for kt in range(4):
                    nc.tensor.matmul(pq[:], w_sb[:, kt, bass.ts(mt, 128)],
                                     xT[:, kt, isl],
                                     start=(kt == 0), stop=(kt == 3))
            if t == 0:
                if eng == 's':
                    nc.scalar.activation(out=q_sb[:, mt, isl], in_=pq[:],
                                         func=AF.Identity,
                                         bias=b_col[:, mt:mt + 1])
                else:
                    nc.vector.tensor_scalar_add(q_sb[:, mt, isl], pq[:],
                                                b_col[:, mt:mt + 1])
            else:
                # K^T: split the head pair into its zero-padded slots
                dsts = (k_sb[0:64, 2 * mt, isl], k_sb[64:128, 2 * mt + 1, isl])
                for hp, dst in enumerate(dsts):
                    psl = slice(64 * hp, 64 * hp + 64)
                    if eng == 's':
                        nc.scalar.activation(out=dst, in_=pq[psl, :],
                                             func=AF.Identity,
                                             bias=b_col[psl, mt:mt + 1])
                    else:
                        nc.vector.tensor_scalar_add(dst, pq[psl, :],
                                                    b_col[psl, mt:mt + 1])

        def attn_head(b, tiles, hh, fill, slots):
            """QK^T + exp for one head.  AV/normalize is NOT emitted here;
            the caller queues it as filler chunks so it drains inside the
            NEXT head's PE gap slots instead of blocking the exp stream at
            the head boundary (the PE executes its queue in order)."""
            _, q_sb, k_sb, v_sb, ot = tiles
            g = hh // 2
            e_t = ep.tile([128, 8, N], FP8, tag="E")
            for jt in range(8):
                pq = psq.tile([128, N], F32, tag="qk")
                for ih in range(2):
                    nc.tensor.matmul(pq[:, bass.ts(ih, 512)],
                                     k_sb[:, hh, bass.ts(jt, 128)],
                                     q_sb[:, g, bass.ts(ih, 512)],
                                     start=True, stop=True)
                nc.scalar.activation(out=e_t[:, jt, :], in_=pq[:],
                                     func=AF.Exp, scale=0.125, bias=ebias[:])
                if jt in slots:
                    fill()
            return e_t

        def av_chunk(b, tiles, hh, ih, e_t, tbb):
            """AV + normalize for one (head, query-half); ~1 PE group."""
            v_sb, ot = tiles[3], tiles[4]
            g = hh // 2
            po = pso.tile([128, 512], F32, tag="o")
            isl = bass.ts(ih, 512)
            # AV DoubleRow pairs; M=65: the ones column of V makes
            # PSUM row 64 the softmax denominator for this half.
            for t in range(4):
                nc.tensor.matmul(po[0:D + 1, :],
                                 v_sb[:, 2 * t:2 * t + 2, hh, 0:D + 1],
                                 e_t[:, 2 * t:2 * t + 2, isl],
                                 start=(t == 0), stop=(t == 3),
                                 perf_mode=DR)
            if NORM_FAST:
                # denominator row -> SBUF (custom-DVE ops misread PSUM at
                # base partition 64), approx reciprocal (~51 ULP), then a
                # gpsimd partition-broadcast: no DMA hops
                s_half = rp.tile([1, 512], F32, tag="s_half", bufs=4)
                nc.vector.tensor_copy(s_half[:], po[D:D + 1, :])
                rs = rp.tile([1, 512], F32, tag="rs", bufs=4)
                nc.vector.reciprocal_approx_fast(out=rs[:], in_=s_half[:])
                rbh = rbp.tile([64, 512], F32, tag="rbh", bufs=4)
                nc.gpsimd.partition_broadcast(rbh[:], rs[:], channels=64)
            else:
                s_half = rp.tile([1, 512], F32, tag="s_half", bufs=4)
                nc.vector.tensor_copy(s_half[:], po[D:D + 1, :])
                # normalize chain: reshape via DRAM, reciprocal on
                # [64, 8], broadcast back
                sdh = drp.tile([512], F32, tag="sdh", bufs=4)
                nc.sync.dma_start(out=sdh[:], in_=s_half[:])
                sph = rp.tile([64, 8], F32, tag="sph", bufs=4)
                nc.sync.dma_start(out=sph[:],
                                  in_=sdh[:].rearrange("(p f) -> p f", p=64))
                rsph = rp.tile([64, 8], F32, tag="rsph", bufs=4)
                nc.vector.reciprocal(out=rsph[:], in_=sph[:])
                rdh = drp.tile([512], F32, tag="rdh", bufs=4)
                nc.sync.dma_start(out=rdh[:].rearrange("(p f) -> p f", p=64),
                                  in_=rsph[:])
                _rdh = rdh[:]
                rbh = rbp.tile([64, 512], F32, tag="rbh", bufs=4)
                nc.sync.dma_start(out=rbh[:], in_=bass.AP(
                    tensor=_rdh.tensor, offset=_rdh.offset,
                    ap=[[0, 64], [1, 512]]))
            if hh % 2 == 0:
                nc.vector.tensor_tensor(ot[0:64, g, isl], po[0:D, :],
                                        rbh[:], op=OP.mult)
            else:
                if ih == 0:
                    tbb[0] = tbp.tile([64, N], O_DT, tag="tb", name="tb")
                nc.vector.tensor_tensor(tbb[0][:, isl], po[0:D, :],
                                        rbh[:], op=OP.mult)
                if ih == 1:
                    nc.sync.dma_start(out=ot[64:128, g, :], in_=tbb[0][:])

        def chunk_outproj(b, tiles, it, dq=None):
            # dq: DMA dispatch queue; phase D uses the scalar queue (idle
            # after the last exp) so the tail's y DMAs don't serialize
            # behind the normalize-chain hops on sync
            dq = dq or nc.sync
            ot = tiles[4]
            py = psf.tile([128, C], F32, tag="f")
            nc.tensor.matmul(py[:], ones128[:], bo_bf[:],
                             start=True, stop=False)
            if FP8_OUT:
                for u in range(2):
                    nc.tensor.matmul(py[:],
                                     ot[:, 2 * u:2 * u + 2, bass.ts(it, 128)],
                                     wo_sb[:, 2 * u:2 * u + 2, :],
                                     start=False, stop=(u == 1),
                                     perf_mode=DR)
            else:
                for g in range(4):
                    nc.tensor.matmul(py[:], ot[:, g, bass.ts(it, 128)],
                                     wo_sb[:, g, :], start=False, stop=(g == 3))
            yt = yp.tile([128, C], F32, tag="y")
            nc.vector.tensor_tensor(yt[:], py[:], x_pref[b][:, it, :],
                              op=OP.add)
            dq.dma_start(out=y_ext[b, bass.ts(it, 128), :], in_=yt[:])

        # ---------- schedule ----------
        # phase A: image-0 transposes (DVE evacs; every scalar op here
        # delays the first exp) + first Q/K group
        for nt in range(8):
            chunk_transpose(0, tiles0, nt, 'v')
        for ih in range(2):
            chunk_qk(0, tiles0, 0, 0, ih, 's')
            chunk_qk(0, tiles0, 0, 1, ih, 's')
        # image-1 x prefetch: dispatched here on the scalar queue so its
        # 2MB transfer starts only after the phase-A critical DMAs
        nc.scalar.dma_start(out=xp1[:],
                            in_=x_ext[1].rearrange("(nt p) c -> p nt c",
                                                   p=128))
        emit_zpads(tiles1[2], nc.scalar)

        # filler units, each ~1 PE accumulation group, drained a few per
        # attention head so projection/AV matmuls fill the PE idle slots
        # left while QK waits on the scalar exp drain (the PE executes
        # its queue in order, so the interleave must be finer than a
        # head).  Queue order encodes the data dependencies:
        #   v0 fully drains in head (0,0)'s 8 slots, before av(0,0) pops;
        #   qk0 m-tile mt drains before attn(0, 2mt) needs it;
        #   T1 -> qk1 -> v1 precede attention(1)/av(1) (FIFO);
        #   av(0,*) jump the queue (appendleft) so e_t tiles recycle
        #   quickly; av(1,*) append behind v1 (they read all of v_sb(1)).
        fillers = deque()
        for it in range(8):
            fillers.append(lambda it=it: chunk_v(0, tiles0, it))
        for mt in range(1, 4):
            for t in range(2):
                for ih in range(2):
                    fillers.append(
                        lambda mt=mt, t=t, ih=ih: chunk_qk(0, tiles0, mt, t, ih, 'v'))
        for nt in range(8):
            fillers.append(
                lambda nt=nt: chunk_transpose(1, tiles1, nt, 'v'))
        for mt in range(4):
            for t in range(2):
                for ih in range(2):
                    fillers.append(
                        lambda mt=mt, t=t, ih=ih: chunk_qk(1, tiles1, mt, t, ih, 'v'))
        for it in range(8):
            fillers.append(lambda it=it: chunk_v(1, tiles1, it))

        def fill():
            if fillers:
                fillers.popleft()()

        tiles_by_b = (tiles0, tiles1)
        ALL_SLOTS = frozenset(range(8))
        MOST_SLOTS = frozenset((1, 2, 3, 5, 6, 7))

        for bb in range(2):
            for hh in range(8):
                if (bb, hh) == (1, 0):
                    # image-0 out-projection drains inside attention(1)
                    for it in range(8):
                        fillers.append(
                            lambda it=it: chunk_outproj(0, tiles0, it))
                slots = (ALL_SLOTS if (bb, hh) in ((0, 0), (1, 6), (1, 7))
                         else MOST_SLOTS)
                e_t = attn_head(bb, tiles_by_b[bb], hh, fill, slots)
                tbb = [None]
                avs = [
                    lambda ih=ih, bb=bb, hh=hh, e_t=e_t, tbb=tbb: av_chunk(
                        bb, tiles_by_b[bb], hh, ih, e_t, tbb)
                    for ih in range(2)]
                if bb == 0 or hh >= 4:
                    # (v_sb(1) is fully drained by image-1 head 3, so
                    # late av chunks can safely jump the queue)
                    fillers.appendleft(avs[1])
                    fillers.appendleft(avs[0])
                else:
                    fillers.extend(avs)
        while fillers:
            fillers.popleft()()

        # phase D: image-1 out-projection
        for it in range(8):
            chunk_outproj(1, tiles1, it, dq=nc.scalar)


def kernel(x, w_qkv, b_qkv, w_out, b_out):

    x = np.ascontiguousarray(np.asarray(x, dtype=np.float32))
    w_qkv = np.ascontiguousarray(np.asarray(w_qkv, dtype=np.float32))
    b_qkv = np.ascontiguousarray(np.asarray(b_qkv, dtype=np.float32))
    w_out = np.ascontiguousarray(np.asarray(w_out, dtype=np.float32))
    b_out = np.ascontiguousarray(np.asarray(b_out, dtype=np.float32))

    bsz, hh, ww, c = x.shape
    assert (bsz, hh, ww, c) == (B, 32, 32, C)
    x_flat = x.reshape(B, N, C)

    if "nc" not in _cache:
        _cache["nc"] = build_nc()
    nc = _cache["nc"]

    if TRACE:
        _register_ntff_hook()

    in_maps = []
    for core in range(NCORES):
        in_maps.append({
            "x": x_flat[NB * core:NB * (core + 1)],
            "w_qkv": w_qkv,
            "b_qkv": b_qkv,
            "w_out": w_out,
            "b_out": b_out,
        })
    res = run_bass_kernel_spmd(nc, in_maps, list(range(NCORES)), trace=TRACE)
    _cache["last_result"] = res
    y = np.concatenate([res.results[i]["y"] for i in range(NCORES)], axis=0)
    return y.reshape(B, 32, 32, C)
